# revision 2
# baseline (speedup 1.0000x reference)
"""NoisyDense forward for Trainium2, 8-core tensor-parallel, bf16+fp8 hybrid.

out = relu(x @ (w_mu + w_sigma * outer(eps_in, eps_out)) + b_mu + b_sigma*eps_out)

Sharding: 2-way over batch x 4-way over units (8 cores).
Per core: x_shard [2048, 4096] (batch rows), w shard [4096, 1024] (unit cols).

Structure (inherited from the bf16 baseline at 267us):
  - Rank-1 factoring: NoisyDense init has row-constant w_sigma, so
    x @ (w_sigma*outer(eps_in,eps_out)) = (x@eps_in) * (sigma*eps_out)^T and
    only x @ w_mu runs on the PE. v = x@eps_in is computed host-side (0.05%
    of FLOPs). If w_sigma is NOT row-constant (never true for the reference
    generator) the host materializes the noisy W and sets u=0.
  - x pre-transposed on host into per-panel lhsT layout; panels 0-1 run as
    an interleaved PAIR so the PE has ~23us of queued work while the w tile
    streams in deadline-ordered chunks; panels 2-15 run solo with x
    prefetched 4 panels ahead (xp bufs=6).

Hybrid precision (the main win over the bf16 baseline): the K=4096
contraction is split
  - K[0:2560)    : bf16 matmuls (20 k-tiles of 128)
  - K[2560:4096) : fp8e4 DoubleRow matmuls (6 instrs of K=256 per
    (panel,ntile)), which measure exactly 2x bf16 PE throughput on HW
    (109.9 vs 215.3 ns per K128xN512 slab; the docs' "Double FP8" mode).
    Operands scaled x*32 / w*8192, clipped to +-240 (TRN E4M3 max), cast
    host-side with ml_dtypes.float8_e4m3; the fp8 psum partial is rescaled
    by 2^-18 and merged during eviction.
  PE work: 26 instead of 32 instr-equivalents per (panel,ntile) = -18.75%
  PE cycles. DMA: x 16.8->13.6MB, w 8.4->6.8MB, out (bf16) 8.4->4.2MB.
  Larger fp8 fractions fail the 2e-2 gate: KO8=14 -> 1.84e-2, KO8=16
  (pure fp8 tail) -> >2e-2. KO8=12 measures rel err 1.707e-2 on HW
  (numpy-emulated prediction 1.702e-2 -- the emulation tracks HW to ~3e-5,
  so the margin is real and deterministic).
  2-pass fp8 splitting schemes (x_hi/x_lo) are pointless on TRN2: at 2x
  rate, 2 fp8 passes cost exactly 1 bf16 pass but add quantization error.

Eviction per panel: z = u*v + b (DVE stt); per 512-half: t = ps8*2^-18 + z
(DVE stt, runs during the panel's bf16 matmuls since the fp8 group stops
first), ot = ps_bf16 + t (DVE add), relu (ScalarE), one [128,1024] bf16
out-DMA. Host upcasts output to fp32. fp8/bf16 parts accumulate in
separate PSUM banks (4 banks per panel, 2-panel pipelining).

Iteration-invariant consts (v, u, b broadcasts) load ONCE before the
chained loop -- reloading them per iteration WAR-serializes a DMA queue on
the previous iteration's last eviction (costs ~3us/iter).

Measured on HW (test.py chained 16-vs-208 loop slope, 8 cores):
  bf16 baseline 267-271us -> hybrid KO8=12: ~217us/iter, rel err 1.707e-2.
  Single-core the same structure runs at the PE floor (~225us bf16 /
  ~181us hybrid); the 8-core residual (~35us) is a DMA-PE interaction that
  microbenches (pure PE 218ns/mm, PE+32MB streaming DMA 227ns/mm, DMA-only
  327GB/s/core) do NOT reproduce -- not raw HBM bandwidth, not DVFS.
  Removing output DMAs or deepening prefetch does not move it.
"""

import numpy as np

BATCH = 4096
IN_DIM = 4096
UNITS = 4096
MSHARDS = 2
NSHARDS = 4
MS = BATCH // MSHARDS      # 2048 rows of x per core
NS = UNITS // NSHARDS      # 1024 units per core
P = 128
KO8 = 12                   # 128-k-tiles computed in fp8 DoubleRow
KOB = IN_DIM // P - KO8    # 24 bf16 k-tiles
JD = KO8 // 2              # DoubleRow instructions per (panel, ntile)
KB = KOB * P               # 3072 bf16 K elements
K8 = KO8 * P               # 1024 fp8 K elements
MP = MS // P               # 16 m-panels per core
NFREE = 512                # one PSUM bank of fp32
NT = NS // NFREE           # 2 n-tiles per core
SX = 32.0                  # fp8 scale for x
SW = 8192.0                # fp8 scale for w
SINV = 1.0 / (SX * SW)     # 2^-18, exact in fp32

_NC_CACHE = {}


def _build(loops=1):
    from concourse import bacc
    import concourse.mybir as mybir
    import concourse.tile as tile

    f32 = mybir.dt.float32
    bf16 = mybir.dt.bfloat16
    fp8 = mybir.dt.float8e4
    DR = mybir.MatmulPerfMode.DoubleRow
    mult = mybir.AluOpType.mult
    add = mybir.AluOpType.add
    relu = mybir.ActivationFunctionType.Relu

    nc = bacc.Bacc(None, target_bir_lowering=False, dynamic_dma_scratch_size=2048)

    # xt_s[pm*128+ki, ko*128+m] = x[pm*128+m, ko*128+ki], ko < KOB (bf16 part)
    xt_d = nc.dram_tensor("xt_s", [MS, KB], bf16, kind="ExternalInput")
    # x8_s[pm*128+p, i*JD*128 + j*128 + m] = q8(x[pm*128+m, KB + j*256 + i*128 + p])
    x8_d = nc.dram_tensor("x8_s", [MS, K8], fp8, kind="ExternalInput")
    # wm_s[ki, ko*NS+n] = w_mu[ko*128+ki, n], ko < KOB
    wm_d = nc.dram_tensor("wm_s", [P, KOB * NS], bf16, kind="ExternalInput")
    # w8_s[p, i*JD*NS + j*NS + n] = q8(w_mu[KB + j*256 + i*128 + p, n])
    w8_d = nc.dram_tensor("w8_s", [P, 2 * JD * NS], fp8, kind="ExternalInput")
    u_d = nc.dram_tensor("u_s", [NS], f32, kind="ExternalInput")     # sigma*eps_out
    b_d = nc.dram_tensor("b_s", [NS], f32, kind="ExternalInput")     # b_mu+b_sig*eps_out
    v_d = nc.dram_tensor("v_s", [MS], f32, kind="ExternalInput")     # x @ eps_in
    out_d = nc.dram_tensor("out_s", [MS, NS], bf16, kind="ExternalOutput")

    with tile.TileContext(nc) as tc:
        with (
            tc.tile_pool(name="const", bufs=1) as const,
            tc.tile_pool(name="wpool", bufs=2) as wpool,
            tc.tile_pool(name="w8pool", bufs=2) as w8pool,
            tc.tile_pool(name="xp", bufs=6) as xp,
            tc.tile_pool(name="x8p", bufs=6) as x8p,
            tc.tile_pool(name="zp", bufs=2) as zp,
            tc.tile_pool(name="tp", bufs=2) as tp,
            tc.tile_pool(name="otp", bufs=2) as otp,
            tc.tile_pool(name="ps", bufs=8, space="PSUM") as psp,
        ):
            v_sb = const.tile([P, MP], f32, tag="vsb")
            u_b = const.tile([P, NS], f32, tag="ub")
            b_b = const.tile([P, NS], f32, tag="bb")
            s_c = const.tile([P, 1], f32, tag="sc")
            nc.any.memset(s_c[:], SINV)

            # iteration-invariant small inputs: load once, before the loop.
            # Reloading them per chained iteration WAR-serializes their DMA
            # queue on the previous iteration's last eviction.
            with nc.allow_non_contiguous_dma(reason="strided/broadcast consts"):
                nc.sync.dma_start(v_sb[:], v_d[:].rearrange("(pm m) -> m pm", m=P))
                nc.sync.dma_start(u_b[:], u_d[None, :].to_broadcast([P, NS]))
                nc.sync.dma_start(b_b[:], b_d[None, :].to_broadcast([P, NS]))

            q = KB // 4  # x quarter (768 cols)

            for _ in range(loops):
                wt = wpool.tile([P, KOB * NS], bf16, tag="w")
                w8t = w8pool.tile([P, 2, JD * NS], fp8, tag="w8")

                def w_kos(a, b):
                    nc.sync.dma_start(wt[:, a * NS : b * NS], wm_d[:, a * NS : b * NS])

                def xpart(xt, pm, a, b):
                    nc.sync.dma_start(xt[:, a:b], xt_d[pm * P : (pm + 1) * P, a:b])

                def x8load(xt8, pm):
                    nc.sync.dma_start(
                        xt8[:],
                        x8_d[pm * P : (pm + 1) * P, :].rearrange(
                            "p (i m) -> p i m", i=2
                        ),
                    )

                # -- head: deadline-ordered stream for the panel-0/1 pair --
                xt0 = xp.tile([P, KB], bf16, tag="xt")
                xt1 = xp.tile([P, KB], bf16, tag="xt")
                xpart(xt0, 0, 0, q)
                xpart(xt1, 1, 0, q)
                wchunks = [(0, 1), (1, 2)] + [
                    (a, min(a + 2, KOB)) for a in range(2, KOB, 2)
                ]
                nq = len(wchunks)
                for idx, (a, b) in enumerate(wchunks):
                    w_kos(a, b)
                    if idx == nq // 4:
                        xpart(xt0, 0, q, 2 * q)
                        xpart(xt1, 1, q, 2 * q)
                    elif idx == nq // 2:
                        xpart(xt0, 0, 2 * q, 3 * q)
                        xpart(xt1, 1, 2 * q, 3 * q)
                    elif idx == 3 * nq // 4:
                        xpart(xt0, 0, 3 * q, KB)
                        xpart(xt1, 1, 3 * q, KB)
                # fp8 weights + pair fp8 x ride behind the bf16 w stream
                nc.sync.dma_start(
                    w8t[:], w8_d[:].rearrange("p (i n) -> p i n", i=2)
                )
                xt8_0 = x8p.tile([P, 2, JD * P], fp8, tag="xt8")
                xt8_1 = x8p.tile([P, 2, JD * P], fp8, tag="xt8")
                x8load(xt8_0, 0)
                x8load(xt8_1, 1)
                # panel 2's data + broadcast constants ride the tail
                xt2 = xp.tile([P, KB], bf16, tag="xt")
                xt8_2 = x8p.tile([P, 2, JD * P], fp8, tag="xt8")
                xpart(xt2, 2, 0, q)
                x8load(xt8_2, 2)

                def w_slice(ko, nt):
                    base = ko * NS + nt * NFREE
                    return wt[:, base : base + NFREE]

                def w8_slice(j, nt):
                    base = j * NS + nt * NFREE
                    return w8t[:, :, base : base + NFREE]

                def dr(ps8, xt8, j, nt, start, stop):
                    nc.tensor.matmul(
                        ps8[:],
                        xt8[:, :, j * P : (j + 1) * P],
                        w8_slice(j, nt),
                        start=start, stop=stop, perf_mode=DR,
                    )

                def evict(pm, psA, psB, ps8A, ps8B):
                    # z = u*v + b on DVE (v precomputed on host)
                    z = zp.tile([P, NS], f32, tag="z")
                    nc.vector.scalar_tensor_tensor(
                        out=z[:], in0=u_b[:], scalar=v_sb[:, pm : pm + 1], in1=b_b[:],
                        op0=mult, op1=add,
                    )
                    # t = ps8 * 2^-18 + z  (can start as soon as the fp8
                    # accumulation group stopped)
                    t = tp.tile([P, NS], f32, tag="t")
                    nc.vector.scalar_tensor_tensor(
                        out=t[:, 0:NFREE], in0=ps8A[:], scalar=s_c[:, 0:1],
                        in1=z[:, 0:NFREE], op0=mult, op1=add,
                    )
                    nc.vector.scalar_tensor_tensor(
                        out=t[:, NFREE:NS], in0=ps8B[:], scalar=s_c[:, 0:1],
                        in1=z[:, NFREE:NS], op0=mult, op1=add,
                    )
                    ot = otp.tile([P, NS], bf16, tag="ot")
                    rows = slice(pm * P, (pm + 1) * P)
                    nc.vector.tensor_add(ot[:, 0:NFREE], psA[:], t[:, 0:NFREE])
                    nc.vector.tensor_add(ot[:, NFREE:NS], psB[:], t[:, NFREE:NS])
                    nc.scalar.activation(ot[:], ot[:], relu)
                    nc.sync.dma_start(out_d[rows, :], ot[:])

                # ---- panels 0-1: interleaved pair (w still streaming) ----
                ps00 = psp.tile([P, NFREE], f32, tag="ps")
                ps01 = psp.tile([P, NFREE], f32, tag="ps")
                ps10 = psp.tile([P, NFREE], f32, tag="ps")
                ps11 = psp.tile([P, NFREE], f32, tag="ps")
                for ko in range(KOB):
                    first = ko == 0
                    last = ko == KOB - 1
                    l0 = xt0[:, ko * P : (ko + 1) * P]
                    l1 = xt1[:, ko * P : (ko + 1) * P]
                    nc.tensor.matmul(ps00[:], l0, w_slice(ko, 0), start=first, stop=last)
                    nc.tensor.matmul(ps01[:], l0, w_slice(ko, 1), start=first, stop=last)
                    nc.tensor.matmul(ps10[:], l1, w_slice(ko, 0), start=first, stop=last)
                    nc.tensor.matmul(ps11[:], l1, w_slice(ko, 1), start=first, stop=last)
                # fp8 DR instrs close out the pair's accumulation (w8 arrived
                # during the bf16 stream)
                ps8_00 = psp.tile([P, NFREE], f32, tag="ps")
                ps8_01 = psp.tile([P, NFREE], f32, tag="ps")
                ps8_10 = psp.tile([P, NFREE], f32, tag="ps")
                ps8_11 = psp.tile([P, NFREE], f32, tag="ps")
                for j in range(JD):
                    first = j == 0
                    last = j == JD - 1
                    dr(ps8_00, xt8_0, j, 0, first, last)
                    dr(ps8_01, xt8_0, j, 1, first, last)
                    dr(ps8_10, xt8_1, j, 0, first, last)
                    dr(ps8_11, xt8_1, j, 1, first, last)
                # finish panel 2, stage panel 3 behind the evictions
                xpart(xt2, 2, q, KB)
                pre_x = {2: (xt2, xt8_2)}
                for pp in (3, 4, 5):
                    xtp = xp.tile([P, KB], bf16, tag="xt")
                    xt8p = x8p.tile([P, 2, JD * P], fp8, tag="xt8")
                    xpart(xtp, pp, 0, KB)
                    x8load(xt8p, pp)
                    pre_x[pp] = (xtp, xt8p)
                evict(0, ps00, ps01, ps8_00, ps8_01)
                evict(1, ps10, ps11, ps8_10, ps8_11)

                # ---- panels 2-15: solo (w resident) ----
                for pm in range(2, MP):
                    xt, xt8 = pre_x.pop(pm)
                    if pm + 4 < MP:
                        nxt = xp.tile([P, KB], bf16, tag="xt")
                        nxt8 = x8p.tile([P, 2, JD * P], fp8, tag="xt8")
                        xpart(nxt, pm + 4, 0, KB)
                        x8load(nxt8, pm + 4)
                        pre_x[pm + 4] = (nxt, nxt8)
                    psA = psp.tile([P, NFREE], f32, tag="ps")
                    psB = psp.tile([P, NFREE], f32, tag="ps")
                    ps8A = psp.tile([P, NFREE], f32, tag="ps")
                    ps8B = psp.tile([P, NFREE], f32, tag="ps")
                    # fp8 groups first: their banks retire early into t
                    for nt, ps8 in ((0, ps8A), (1, ps8B)):
                        for j in range(JD):
                            dr(ps8, xt8, j, nt, j == 0, j == JD - 1)
                    for nt, ps in ((0, psA), (1, psB)):
                        for ko in range(KOB):
                            lh = xt[:, ko * P : (ko + 1) * P]
                            nc.tensor.matmul(
                                ps[:], lh, w_slice(ko, nt),
                                start=(ko == 0), stop=(ko == KOB - 1),
                            )
                    evict(pm, psA, psB, ps8A, ps8B)

    nc.compile()
    return nc


def get_nc(variant="rank1", loops=1):
    key = loops
    if key not in _NC_CACHE:
        _NC_CACHE[key] = _build(loops)
    return _NC_CACHE[key]


def pick_variant(w_sigma):
    w_sigma = np.asarray(w_sigma)
    return "rank1" if bool((w_sigma == w_sigma[0:1, :]).all()) else "general"


def _to_bf16(a):
    import ml_dtypes

    return np.ascontiguousarray(a).astype(ml_dtypes.bfloat16)


def _to_fp8(a, scale):
    import ml_dtypes

    s = np.clip(np.asarray(a, dtype=np.float32) * scale, -240.0, 240.0)
    return np.ascontiguousarray(s).astype(ml_dtypes.float8_e4m3)


def _xt_layout(xs):
    # [MS, KB] -> xt[pm*128+ki, ko*128+m] = xs[pm*128+m, ko*128+ki]
    a = xs.reshape(MP, P, KOB, P)          # [pm, m, ko, ki]
    return a.transpose(0, 3, 2, 1).reshape(MS, KB)


def _x8_layout(xs8):
    # [MS, K8] (fp8 values) -> x8[pm*128+p, i*JD*128 + j*128 + m]
    #   = xs8[pm*128+m, j*256 + i*128 + p]
    a = xs8.reshape(MP, P, JD, 2, P)       # [pm, m, j, i, p]
    return a.transpose(0, 4, 3, 2, 1).reshape(MS, K8)


def _w_layout(ws):
    # [KB, NS] -> wm[ki, ko*NS+n] = ws[ko*128+ki, n]
    return ws.reshape(KOB, P, NS).transpose(1, 0, 2).reshape(P, KOB * NS)


def _w8_layout(ws8):
    # [K8, NS] (fp8 values) -> w8[p, i*JD*NS + j*NS + n] = ws8[j*256+i*128+p, n]
    a = ws8.reshape(JD, 2, P, NS)          # [j, i, p, n]
    return a.transpose(2, 1, 0, 3).reshape(P, 2 * JD * NS)


def shard_inputs(x, w_mu, w_sigma, b_mu, b_sigma, eps_in, eps_out, variant="rank1"):
    x = np.asarray(x, dtype=np.float32)
    w_mu = np.asarray(w_mu, dtype=np.float32)
    w_sigma = np.asarray(w_sigma, dtype=np.float32)
    b_mu = np.asarray(b_mu, dtype=np.float32)
    b_sigma = np.asarray(b_sigma, dtype=np.float32)
    eps_in = np.asarray(eps_in, dtype=np.float32)
    eps_out = np.asarray(eps_out, dtype=np.float32)

    # v = x @ eps_in per batch row-group (tiny rank-1 preprocessing)
    vs = [
        np.ascontiguousarray(x[mr * MS : (mr + 1) * MS, :] @ eps_in, dtype=np.float32)
        for mr in range(MSHARDS)
    ]
    # one pre-transposed bf16 x + one fp8 x tail per batch row-group,
    # shared by 4 cores each
    xts = [
        _to_bf16(_xt_layout(x[mr * MS : (mr + 1) * MS, 0:KB]))
        for mr in range(MSHARDS)
    ]
    x8s = [
        _x8_layout(_to_fp8(x[mr * MS : (mr + 1) * MS, KB:IN_DIM], SX))
        for mr in range(MSHARDS)
    ]

    in_maps = []
    for c in range(MSHARDS * NSHARDS):
        mr, ncol = divmod(c, NSHARDS)
        nsl = slice(ncol * NS, (ncol + 1) * NS)
        if variant == "rank1":
            wshard = w_mu[:, nsl]
            u = w_sigma[0, nsl] * eps_out[nsl]
        else:
            # general fallback: materialize noisy W on host, disable rank-1 term
            wshard = w_mu[:, nsl] + w_sigma[:, nsl] * (
                eps_in[:, None] * eps_out[None, nsl]
            )
            u = np.zeros(NS, dtype=np.float32)
        m = {
            "xt_s": xts[mr],
            "x8_s": x8s[mr],
            "wm_s": _to_bf16(_w_layout(wshard[0:KB, :])),
            "w8_s": _w8_layout(_to_fp8(wshard[KB:IN_DIM, :], SW)),
            "u_s": np.ascontiguousarray(u, dtype=np.float32),
            "b_s": np.ascontiguousarray(
                b_mu[nsl] + b_sigma[nsl] * eps_out[nsl], dtype=np.float32
            ),
            "v_s": vs[mr],
        }
        in_maps.append(m)
    return in_maps


def unshard_output(results):
    out = np.empty((BATCH, UNITS), dtype=np.float32)
    for c, rmap in enumerate(results):
        mr, ncol = divmod(c, NSHARDS)
        out[mr * MS : (mr + 1) * MS, ncol * NS : (ncol + 1) * NS] = np.asarray(
            rmap["out_s"]
        ).astype(np.float32)
    return out


def kernel(x, w_mu, w_sigma, b_mu, b_sigma, eps_in, eps_out):
    from concourse.bass_utils import run_bass_kernel_spmd

    variant = pick_variant(w_sigma)
    nc = get_nc(variant)
    in_maps = shard_inputs(
        x, w_mu, w_sigma, b_mu, b_sigma, eps_in, eps_out, variant=variant
    )
    res = run_bass_kernel_spmd(nc, in_maps, core_ids=list(range(8)))
    return unshard_output(res.results)


# revision 3
# speedup vs baseline: 1.0025x; 1.0025x over previous
"""NoisyDense forward for Trainium2, 8-core tensor-parallel, bf16+fp8 hybrid.

out = relu(x @ (w_mu + w_sigma * outer(eps_in, eps_out)) + b_mu + b_sigma*eps_out)

Sharding: 2-way over batch x 4-way over units (8 cores).
Per core: x_shard [2048, 4096] (batch rows), w shard [4096, 1024] (unit cols).

Structure (inherited from the bf16 baseline at 267us):
  - Rank-1 factoring: NoisyDense init has row-constant w_sigma, so
    x @ (w_sigma*outer(eps_in,eps_out)) = (x@eps_in) * (sigma*eps_out)^T and
    only x @ w_mu runs on the PE. v = x@eps_in is computed host-side (0.05%
    of FLOPs). If w_sigma is NOT row-constant (never true for the reference
    generator) the host materializes the noisy W and sets u=0.
  - x pre-transposed on host into per-panel lhsT layout; panels 0-1 run as
    an interleaved PAIR so the PE has ~23us of queued work while the w tile
    streams in deadline-ordered chunks; panels 2-15 run solo with x
    prefetched 4 panels ahead (xp bufs=6).

Hybrid precision (the main win over the bf16 baseline): the K=4096
contraction is split
  - K[0:2560)    : bf16 matmuls (20 k-tiles of 128)
  - K[2560:4096) : fp8e4 DoubleRow matmuls (6 instrs of K=256 per
    (panel,ntile)), which measure exactly 2x bf16 PE throughput on HW
    (109.9 vs 215.3 ns per K128xN512 slab; the docs' "Double FP8" mode).
    Operands scaled x*32 / w*8192, clipped to +-240 (TRN E4M3 max), cast
    host-side with ml_dtypes.float8_e4m3; the fp8 psum partial is rescaled
    by 2^-18 and merged during eviction.
  PE work: 26 instead of 32 instr-equivalents per (panel,ntile) = -18.75%
  PE cycles. DMA: x 16.8->13.6MB, w 8.4->6.8MB, out (bf16) 8.4->4.2MB.
  Larger fp8 fractions fail the 2e-2 gate: KO8=14 -> 1.84e-2, KO8=16
  (pure fp8 tail) -> >2e-2. KO8=12 measures rel err 1.707e-2 on HW
  (numpy-emulated prediction 1.702e-2 -- the emulation tracks HW to ~3e-5,
  so the margin is real and deterministic).
  2-pass fp8 splitting schemes (x_hi/x_lo) are pointless on TRN2: at 2x
  rate, 2 fp8 passes cost exactly 1 bf16 pass but add quantization error.

Eviction per panel: z = u*v + b (DVE stt); per 512-half: t = ps8*2^-18 + z
(DVE stt, runs during the panel's bf16 matmuls since the fp8 group stops
first), ot = ps_bf16 + t (DVE add), relu (ScalarE), one [128,1024] bf16
out-DMA. Host upcasts output to fp32. fp8/bf16 parts accumulate in
separate PSUM banks (4 banks per panel, 2-panel pipelining).

Iteration-invariant consts (v, u, b broadcasts) load ONCE before the
chained loop -- reloading them per iteration WAR-serializes a DMA queue on
the previous iteration's last eviction (costs ~3us/iter).

Measured on HW (test.py chained 16-vs-208 loop slope, 8 cores):
  bf16 baseline 267-271us -> hybrid KO8=12: ~217us/iter, rel err 1.707e-2.
  Single-core the same structure runs at the PE floor (~225us bf16 /
  ~181us hybrid); the 8-core residual (~35us) is a DMA-PE interaction that
  microbenches (pure PE 218ns/mm, PE+32MB streaming DMA 227ns/mm, DMA-only
  327GB/s/core) do NOT reproduce -- not raw HBM bandwidth, not DVFS.
  Removing output DMAs or deepening prefetch does not move it.
"""

import numpy as np

BATCH = 4096
IN_DIM = 4096
UNITS = 4096
MSHARDS = 2
NSHARDS = 4
MS = BATCH // MSHARDS      # 2048 rows of x per core
NS = UNITS // NSHARDS      # 1024 units per core
P = 128
KO8 = 12                   # 128-k-tiles computed in fp8 DoubleRow
KOB = IN_DIM // P - KO8    # 24 bf16 k-tiles
JD = KO8 // 2              # DoubleRow instructions per (panel, ntile)
KB = KOB * P               # 3072 bf16 K elements
K8 = KO8 * P               # 1024 fp8 K elements
MP = MS // P               # 16 m-panels per core
NFREE = 512                # one PSUM bank of fp32
NT = NS // NFREE           # 2 n-tiles per core
SX = 32.0                  # fp8 scale for x
SW = 8192.0                # fp8 scale for w_mu (rank1 variant; |w_mu|<=2^-6)
SW_GEN = 2048.0            # fp8 scale for the materialized noisy W (general
                           # variant; |W| can reach ~0.06, keep under 240)

_NC_CACHE = {}


def _build(loops=1, sw=SW):
    sinv = 1.0 / (SX * sw)  # power of 2, exact in fp32
    from concourse import bacc
    import concourse.mybir as mybir
    import concourse.tile as tile

    f32 = mybir.dt.float32
    bf16 = mybir.dt.bfloat16
    fp8 = mybir.dt.float8e4
    DR = mybir.MatmulPerfMode.DoubleRow
    mult = mybir.AluOpType.mult
    add = mybir.AluOpType.add
    relu = mybir.ActivationFunctionType.Relu

    nc = bacc.Bacc(None, target_bir_lowering=False, dynamic_dma_scratch_size=2048)

    # xt_s[pm*128+ki, ko*128+m] = x[pm*128+m, ko*128+ki], ko < KOB (bf16 part)
    xt_d = nc.dram_tensor("xt_s", [MS, KB], bf16, kind="ExternalInput")
    # x8_s[pm*128+p, i*JD*128 + j*128 + m] = q8(x[pm*128+m, KB + j*256 + i*128 + p])
    x8_d = nc.dram_tensor("x8_s", [MS, K8], fp8, kind="ExternalInput")
    # wm_s[ki, ko*NS+n] = w_mu[ko*128+ki, n], ko < KOB
    wm_d = nc.dram_tensor("wm_s", [P, KOB * NS], bf16, kind="ExternalInput")
    # w8_s[p, i*JD*NS + j*NS + n] = q8(w_mu[KB + j*256 + i*128 + p, n])
    w8_d = nc.dram_tensor("w8_s", [P, 2 * JD * NS], fp8, kind="ExternalInput")
    u_d = nc.dram_tensor("u_s", [NS], f32, kind="ExternalInput")     # sigma*eps_out
    b_d = nc.dram_tensor("b_s", [NS], f32, kind="ExternalInput")     # b_mu+b_sig*eps_out
    v_d = nc.dram_tensor("v_s", [MS], f32, kind="ExternalInput")     # x @ eps_in
    out_d = nc.dram_tensor("out_s", [MS, NS], bf16, kind="ExternalOutput")

    with tile.TileContext(nc) as tc:
        with (
            tc.tile_pool(name="const", bufs=1) as const,
            tc.tile_pool(name="wpool", bufs=2) as wpool,
            tc.tile_pool(name="w8pool", bufs=2) as w8pool,
            tc.tile_pool(name="xp", bufs=6) as xp,
            tc.tile_pool(name="x8p", bufs=6) as x8p,
            tc.tile_pool(name="zp", bufs=2) as zp,
            tc.tile_pool(name="tp", bufs=2) as tp,
            tc.tile_pool(name="otp", bufs=2) as otp,
            tc.tile_pool(name="ps", bufs=8, space="PSUM") as psp,
        ):
            v_sb = const.tile([P, MP], f32, tag="vsb")
            u_b = const.tile([P, NS], f32, tag="ub")
            b_b = const.tile([P, NS], f32, tag="bb")
            s_c = const.tile([P, 1], f32, tag="sc")
            nc.any.memset(s_c[:], sinv)

            # iteration-invariant small inputs: load once, before the loop.
            # Reloading them per chained iteration WAR-serializes their DMA
            # queue on the previous iteration's last eviction.
            with nc.allow_non_contiguous_dma(reason="strided/broadcast consts"):
                nc.sync.dma_start(v_sb[:], v_d[:].rearrange("(pm m) -> m pm", m=P))
                nc.sync.dma_start(u_b[:], u_d[None, :].to_broadcast([P, NS]))
                nc.sync.dma_start(b_b[:], b_d[None, :].to_broadcast([P, NS]))

            q = KB // 4  # x quarter (768 cols)

            for _ in range(loops):
                wt = wpool.tile([P, KOB * NS], bf16, tag="w")
                w8t = w8pool.tile([P, 2, JD * NS], fp8, tag="w8")

                def w_kos(a, b):
                    nc.sync.dma_start(wt[:, a * NS : b * NS], wm_d[:, a * NS : b * NS])

                def xpart(xt, pm, a, b):
                    nc.sync.dma_start(xt[:, a:b], xt_d[pm * P : (pm + 1) * P, a:b])

                def x8load(xt8, pm):
                    nc.sync.dma_start(
                        xt8[:],
                        x8_d[pm * P : (pm + 1) * P, :].rearrange(
                            "p (i m) -> p i m", i=2
                        ),
                    )

                # -- head: deadline-ordered stream for the panel-0/1 pair --
                xt0 = xp.tile([P, KB], bf16, tag="xt")
                xt1 = xp.tile([P, KB], bf16, tag="xt")
                xpart(xt0, 0, 0, q)
                xpart(xt1, 1, 0, q)
                wchunks = [(0, 1), (1, 2)] + [
                    (a, min(a + 2, KOB)) for a in range(2, KOB, 2)
                ]
                nq = len(wchunks)
                for idx, (a, b) in enumerate(wchunks):
                    w_kos(a, b)
                    if idx == nq // 4:
                        xpart(xt0, 0, q, 2 * q)
                        xpart(xt1, 1, q, 2 * q)
                    elif idx == nq // 2:
                        xpart(xt0, 0, 2 * q, 3 * q)
                        xpart(xt1, 1, 2 * q, 3 * q)
                    elif idx == 3 * nq // 4:
                        xpart(xt0, 0, 3 * q, KB)
                        xpart(xt1, 1, 3 * q, KB)
                # fp8 weights + pair fp8 x ride behind the bf16 w stream
                nc.sync.dma_start(
                    w8t[:], w8_d[:].rearrange("p (i n) -> p i n", i=2)
                )
                xt8_0 = x8p.tile([P, 2, JD * P], fp8, tag="xt8")
                xt8_1 = x8p.tile([P, 2, JD * P], fp8, tag="xt8")
                x8load(xt8_0, 0)
                x8load(xt8_1, 1)
                # panel 2's data + broadcast constants ride the tail
                xt2 = xp.tile([P, KB], bf16, tag="xt")
                xt8_2 = x8p.tile([P, 2, JD * P], fp8, tag="xt8")
                xpart(xt2, 2, 0, q)
                x8load(xt8_2, 2)

                def w_slice(ko, nt):
                    base = ko * NS + nt * NFREE
                    return wt[:, base : base + NFREE]

                def w8_slice(j, nt):
                    base = j * NS + nt * NFREE
                    return w8t[:, :, base : base + NFREE]

                def dr(ps8, xt8, j, nt, start, stop):
                    nc.tensor.matmul(
                        ps8[:],
                        xt8[:, :, j * P : (j + 1) * P],
                        w8_slice(j, nt),
                        start=start, stop=stop, perf_mode=DR,
                    )

                def evict(pm, psA, psB, ps8A, ps8B):
                    # z = u*v + b on DVE (v precomputed on host)
                    z = zp.tile([P, NS], f32, tag="z")
                    nc.vector.scalar_tensor_tensor(
                        out=z[:], in0=u_b[:], scalar=v_sb[:, pm : pm + 1], in1=b_b[:],
                        op0=mult, op1=add,
                    )
                    # t = ps8 * 2^-18 + z  (can start as soon as the fp8
                    # accumulation group stopped)
                    t = tp.tile([P, NS], f32, tag="t")
                    nc.vector.scalar_tensor_tensor(
                        out=t[:, 0:NFREE], in0=ps8A[:], scalar=s_c[:, 0:1],
                        in1=z[:, 0:NFREE], op0=mult, op1=add,
                    )
                    nc.vector.scalar_tensor_tensor(
                        out=t[:, NFREE:NS], in0=ps8B[:], scalar=s_c[:, 0:1],
                        in1=z[:, NFREE:NS], op0=mult, op1=add,
                    )
                    ot = otp.tile([P, NS], bf16, tag="ot")
                    rows = slice(pm * P, (pm + 1) * P)
                    nc.vector.tensor_add(ot[:, 0:NFREE], psA[:], t[:, 0:NFREE])
                    nc.vector.tensor_add(ot[:, NFREE:NS], psB[:], t[:, NFREE:NS])
                    nc.scalar.activation(ot[:], ot[:], relu)
                    nc.sync.dma_start(out_d[rows, :], ot[:])

                # ---- panels 0-1: interleaved pair (w still streaming) ----
                ps00 = psp.tile([P, NFREE], f32, tag="ps")
                ps01 = psp.tile([P, NFREE], f32, tag="ps")
                ps10 = psp.tile([P, NFREE], f32, tag="ps")
                ps11 = psp.tile([P, NFREE], f32, tag="ps")
                for ko in range(KOB):
                    first = ko == 0
                    last = ko == KOB - 1
                    l0 = xt0[:, ko * P : (ko + 1) * P]
                    l1 = xt1[:, ko * P : (ko + 1) * P]
                    nc.tensor.matmul(ps00[:], l0, w_slice(ko, 0), start=first, stop=last)
                    nc.tensor.matmul(ps01[:], l0, w_slice(ko, 1), start=first, stop=last)
                    nc.tensor.matmul(ps10[:], l1, w_slice(ko, 0), start=first, stop=last)
                    nc.tensor.matmul(ps11[:], l1, w_slice(ko, 1), start=first, stop=last)
                # fp8 DR instrs close out the pair's accumulation (w8 arrived
                # during the bf16 stream)
                ps8_00 = psp.tile([P, NFREE], f32, tag="ps")
                ps8_01 = psp.tile([P, NFREE], f32, tag="ps")
                ps8_10 = psp.tile([P, NFREE], f32, tag="ps")
                ps8_11 = psp.tile([P, NFREE], f32, tag="ps")
                for j in range(JD):
                    first = j == 0
                    last = j == JD - 1
                    dr(ps8_00, xt8_0, j, 0, first, last)
                    dr(ps8_01, xt8_0, j, 1, first, last)
                    dr(ps8_10, xt8_1, j, 0, first, last)
                    dr(ps8_11, xt8_1, j, 1, first, last)
                # finish panel 2, stage panel 3 behind the evictions
                xpart(xt2, 2, q, KB)
                pre_x = {2: (xt2, xt8_2)}
                for pp in (3, 4, 5):
                    xtp = xp.tile([P, KB], bf16, tag="xt")
                    xt8p = x8p.tile([P, 2, JD * P], fp8, tag="xt8")
                    xpart(xtp, pp, 0, KB)
                    x8load(xt8p, pp)
                    pre_x[pp] = (xtp, xt8p)
                evict(0, ps00, ps01, ps8_00, ps8_01)
                evict(1, ps10, ps11, ps8_10, ps8_11)

                # ---- panels 2-15: solo (w resident) ----
                for pm in range(2, MP):
                    xt, xt8 = pre_x.pop(pm)
                    if pm + 4 < MP:
                        nxt = xp.tile([P, KB], bf16, tag="xt")
                        nxt8 = x8p.tile([P, 2, JD * P], fp8, tag="xt8")
                        xpart(nxt, pm + 4, 0, KB)
                        x8load(nxt8, pm + 4)
                        pre_x[pm + 4] = (nxt, nxt8)
                    psA = psp.tile([P, NFREE], f32, tag="ps")
                    psB = psp.tile([P, NFREE], f32, tag="ps")
                    ps8A = psp.tile([P, NFREE], f32, tag="ps")
                    ps8B = psp.tile([P, NFREE], f32, tag="ps")
                    # fp8 groups first: their banks retire early into t
                    for nt, ps8 in ((0, ps8A), (1, ps8B)):
                        for j in range(JD):
                            dr(ps8, xt8, j, nt, j == 0, j == JD - 1)
                    for nt, ps in ((0, psA), (1, psB)):
                        for ko in range(KOB):
                            lh = xt[:, ko * P : (ko + 1) * P]
                            nc.tensor.matmul(
                                ps[:], lh, w_slice(ko, nt),
                                start=(ko == 0), stop=(ko == KOB - 1),
                            )
                    evict(pm, psA, psB, ps8A, ps8B)

    nc.compile()
    return nc


def get_nc(variant="rank1", loops=1):
    sw = SW if variant == "rank1" else SW_GEN
    key = (loops, sw)
    if key not in _NC_CACHE:
        _NC_CACHE[key] = _build(loops, sw)
    return _NC_CACHE[key]


def pick_variant(w_sigma):
    w_sigma = np.asarray(w_sigma)
    return "rank1" if bool((w_sigma == w_sigma[0:1, :]).all()) else "general"


def _to_bf16(a):
    import ml_dtypes

    return np.ascontiguousarray(a).astype(ml_dtypes.bfloat16)


def _to_fp8(a, scale):
    import ml_dtypes

    s = np.clip(np.asarray(a, dtype=np.float32) * scale, -240.0, 240.0)
    return np.ascontiguousarray(s).astype(ml_dtypes.float8_e4m3)


def _xt_layout(xs):
    # [MS, KB] -> xt[pm*128+ki, ko*128+m] = xs[pm*128+m, ko*128+ki]
    a = xs.reshape(MP, P, KOB, P)          # [pm, m, ko, ki]
    return a.transpose(0, 3, 2, 1).reshape(MS, KB)


def _x8_layout(xs8):
    # [MS, K8] (fp8 values) -> x8[pm*128+p, i*JD*128 + j*128 + m]
    #   = xs8[pm*128+m, j*256 + i*128 + p]
    a = xs8.reshape(MP, P, JD, 2, P)       # [pm, m, j, i, p]
    return a.transpose(0, 4, 3, 2, 1).reshape(MS, K8)


def _w_layout(ws):
    # [KB, NS] -> wm[ki, ko*NS+n] = ws[ko*128+ki, n]
    return ws.reshape(KOB, P, NS).transpose(1, 0, 2).reshape(P, KOB * NS)


def _w8_layout(ws8):
    # [K8, NS] (fp8 values) -> w8[p, i*JD*NS + j*NS + n] = ws8[j*256+i*128+p, n]
    a = ws8.reshape(JD, 2, P, NS)          # [j, i, p, n]
    return a.transpose(2, 1, 0, 3).reshape(P, 2 * JD * NS)


def shard_inputs(x, w_mu, w_sigma, b_mu, b_sigma, eps_in, eps_out, variant="rank1"):
    x = np.asarray(x, dtype=np.float32)
    w_mu = np.asarray(w_mu, dtype=np.float32)
    w_sigma = np.asarray(w_sigma, dtype=np.float32)
    b_mu = np.asarray(b_mu, dtype=np.float32)
    b_sigma = np.asarray(b_sigma, dtype=np.float32)
    eps_in = np.asarray(eps_in, dtype=np.float32)
    eps_out = np.asarray(eps_out, dtype=np.float32)

    # v = x @ eps_in per batch row-group (tiny rank-1 preprocessing)
    vs = [
        np.ascontiguousarray(x[mr * MS : (mr + 1) * MS, :] @ eps_in, dtype=np.float32)
        for mr in range(MSHARDS)
    ]
    # one pre-transposed bf16 x + one fp8 x tail per batch row-group,
    # shared by 4 cores each
    xts = [
        _to_bf16(_xt_layout(x[mr * MS : (mr + 1) * MS, 0:KB]))
        for mr in range(MSHARDS)
    ]
    x8s = [
        _x8_layout(_to_fp8(x[mr * MS : (mr + 1) * MS, KB:IN_DIM], SX))
        for mr in range(MSHARDS)
    ]

    in_maps = []
    for c in range(MSHARDS * NSHARDS):
        mr, ncol = divmod(c, NSHARDS)
        nsl = slice(ncol * NS, (ncol + 1) * NS)
        if variant == "rank1":
            wshard = w_mu[:, nsl]
            u = w_sigma[0, nsl] * eps_out[nsl]
            sw = SW
        else:
            # general fallback: materialize noisy W on host, disable rank-1 term
            wshard = w_mu[:, nsl] + w_sigma[:, nsl] * (
                eps_in[:, None] * eps_out[None, nsl]
            )
            u = np.zeros(NS, dtype=np.float32)
            sw = SW_GEN
        m = {
            "xt_s": xts[mr],
            "x8_s": x8s[mr],
            "wm_s": _to_bf16(_w_layout(wshard[0:KB, :])),
            "w8_s": _w8_layout(_to_fp8(wshard[KB:IN_DIM, :], sw)),
            "u_s": np.ascontiguousarray(u, dtype=np.float32),
            "b_s": np.ascontiguousarray(
                b_mu[nsl] + b_sigma[nsl] * eps_out[nsl], dtype=np.float32
            ),
            "v_s": vs[mr],
        }
        in_maps.append(m)
    return in_maps


def unshard_output(results):
    out = np.empty((BATCH, UNITS), dtype=np.float32)
    for c, rmap in enumerate(results):
        mr, ncol = divmod(c, NSHARDS)
        out[mr * MS : (mr + 1) * MS, ncol * NS : (ncol + 1) * NS] = np.asarray(
            rmap["out_s"]
        ).astype(np.float32)
    return out


def kernel(x, w_mu, w_sigma, b_mu, b_sigma, eps_in, eps_out):
    from concourse.bass_utils import run_bass_kernel_spmd

    variant = pick_variant(w_sigma)
    nc = get_nc(variant)
    in_maps = shard_inputs(
        x, w_mu, w_sigma, b_mu, b_sigma, eps_in, eps_out, variant=variant
    )
    res = run_bass_kernel_spmd(nc, in_maps, core_ids=list(range(8)))
    return unshard_output(res.results)


# revision 4
# speedup vs baseline: 1.0232x; 1.0206x over previous
"""NoisyDense forward for Trainium2, 8-core tensor-parallel, bf16+fp8 hybrid.

out = relu(x @ (w_mu + w_sigma * outer(eps_in, eps_out)) + b_mu + b_sigma*eps_out)

Sharding: 2-way over batch x 4-way over units (8 cores).
Per core: x_shard [2048, 4096] (batch rows), w shard [4096, 1024] (unit cols).

Structure (inherited from the bf16 baseline at 267us):
  - Rank-1 factoring: NoisyDense init has row-constant w_sigma, so
    x @ (w_sigma*outer(eps_in,eps_out)) = (x@eps_in) * (sigma*eps_out)^T and
    only x @ w_mu runs on the PE. v = x@eps_in is computed host-side (0.05%
    of FLOPs). If w_sigma is NOT row-constant (never true for the reference
    generator) the host materializes the noisy W and sets u=0.
  - x pre-transposed on host into per-panel lhsT layout; panels 0-1 run as
    an interleaved PAIR so the PE has ~23us of queued work while the w tile
    streams in deadline-ordered chunks; panels 2-15 run solo with x
    prefetched 4 panels ahead (xp bufs=6).

Hybrid precision (the main win over the bf16 baseline): the K=4096
contraction is split
  - K[0:2560)    : bf16 matmuls (20 k-tiles of 128)
  - K[2560:4096) : fp8e4 DoubleRow matmuls (6 instrs of K=256 per
    (panel,ntile)), which measure exactly 2x bf16 PE throughput on HW
    (109.9 vs 215.3 ns per K128xN512 slab; the docs' "Double FP8" mode).
    Operands scaled x*32 / w*8192, clipped to +-240 (TRN E4M3 max), cast
    host-side with ml_dtypes.float8_e4m3; the fp8 psum partial is rescaled
    by 2^-18 and merged during eviction.
  PE work: 26 instead of 32 instr-equivalents per (panel,ntile) = -18.75%
  PE cycles. DMA: x 16.8->13.6MB, w 8.4->6.8MB, out (bf16) 8.4->4.2MB.
  Larger fp8 fractions fail the 2e-2 gate: KO8=14 -> 1.84e-2, KO8=16
  (pure fp8 tail) -> >2e-2. KO8=12 measures rel err 1.707e-2 on HW
  (numpy-emulated prediction 1.702e-2 -- the emulation tracks HW to ~3e-5,
  so the margin is real and deterministic).
  2-pass fp8 splitting schemes (x_hi/x_lo) are pointless on TRN2: at 2x
  rate, 2 fp8 passes cost exactly 1 bf16 pass but add quantization error.

Eviction per panel: z = u*v + b (DVE stt); per 512-half: t = ps8*2^-18 + z
(DVE stt, runs during the panel's bf16 matmuls since the fp8 group stops
first), ot = ps_bf16 + t (DVE add), relu (ScalarE), one [128,1024] bf16
out-DMA. Host upcasts output to fp32. fp8/bf16 parts accumulate in
separate PSUM banks (4 banks per panel, 2-panel pipelining).

Iteration-invariant consts (v, u, b broadcasts) load ONCE before the
chained loop -- reloading them per iteration WAR-serializes a DMA queue on
the previous iteration's last eviction (costs ~3us/iter).

Measured on HW (test.py chained 16-vs-208 loop slope, 8 cores):
  bf16 baseline 267-271us -> hybrid KO8=12: ~217us/iter, rel err 1.707e-2.
  Single-core the same structure runs at the PE floor (~225us bf16 /
  ~181us hybrid); the 8-core residual (~35us) is a DMA-PE interaction that
  microbenches (pure PE 218ns/mm, PE+32MB streaming DMA 227ns/mm, DMA-only
  327GB/s/core) do NOT reproduce -- not raw HBM bandwidth, not DVFS.
  Also tested and NEUTRAL on HW: removing output DMAs; prefetch depth 2
  vs 4; cross-iteration stream-ahead of the next iteration's w/x spread
  over the solo panels (kills the per-iteration head burst -- no change);
  stationary run-length 2 (ko-outer/nt-inner). Run-length-4 microbenches
  at only ~211 vs 215-218 ns/mm, so a w-stationary layout (transposed
  output) is not worth it. An earlier 164ns/mm same-stationary reading
  was an artifact of start/stop-per-matmul mode.
"""

import numpy as np

BATCH = 4096
IN_DIM = 4096
UNITS = 4096
MSHARDS = 2
NSHARDS = 4
MS = BATCH // MSHARDS      # 2048 rows of x per core
NS = UNITS // NSHARDS      # 1024 units per core
P = 128
KO8 = 12                   # 128-k-tiles computed in fp8 DoubleRow
KOB = IN_DIM // P - KO8    # 24 bf16 k-tiles
JD = KO8 // 2              # DoubleRow instructions per (panel, ntile)
KB = KOB * P               # 3072 bf16 K elements
K8 = KO8 * P               # 1024 fp8 K elements
MP = MS // P               # 16 m-panels per core
NFREE = 512                # one PSUM bank of fp32
NT = NS // NFREE           # 2 n-tiles per core
SX = 32.0                  # fp8 scale for x
SW = 8192.0                # fp8 scale for w_mu (rank1 variant; |w_mu|<=2^-6)
SW_GEN = 2048.0            # fp8 scale for the materialized noisy W (general
                           # variant; |W| can reach ~0.06, keep under 240)

_NC_CACHE = {}


def _build(loops=1, sw=SW):
    sinv = 1.0 / (SX * sw)  # power of 2, exact in fp32
    from concourse import bacc
    import concourse.mybir as mybir
    import concourse.tile as tile

    f32 = mybir.dt.float32
    bf16 = mybir.dt.bfloat16
    fp8 = mybir.dt.float8e4
    DR = mybir.MatmulPerfMode.DoubleRow
    mult = mybir.AluOpType.mult
    add = mybir.AluOpType.add
    relu = mybir.ActivationFunctionType.Relu

    nc = bacc.Bacc(None, target_bir_lowering=False, dynamic_dma_scratch_size=2048)

    # xt_s[pm*128+ki, ko*128+m] = x[pm*128+m, ko*128+ki], ko < KOB (bf16 part)
    xt_d = nc.dram_tensor("xt_s", [MS, KB], bf16, kind="ExternalInput")
    # x8_s[pm*128+p, i*JD*128 + j*128 + m] = q8(x[pm*128+m, KB + j*256 + i*128 + p])
    x8_d = nc.dram_tensor("x8_s", [MS, K8], fp8, kind="ExternalInput")
    # wm_s[ki, ko*NS+n] = w_mu[ko*128+ki, n], ko < KOB
    wm_d = nc.dram_tensor("wm_s", [P, KOB * NS], bf16, kind="ExternalInput")
    # w8_s[p, i*JD*NS + j*NS + n] = q8(w_mu[KB + j*256 + i*128 + p, n])
    w8_d = nc.dram_tensor("w8_s", [P, 2 * JD * NS], fp8, kind="ExternalInput")
    u_d = nc.dram_tensor("u_s", [NS], f32, kind="ExternalInput")     # sigma*eps_out
    b_d = nc.dram_tensor("b_s", [NS], f32, kind="ExternalInput")     # b_mu+b_sig*eps_out
    v_d = nc.dram_tensor("v_s", [MS], f32, kind="ExternalInput")     # x @ eps_in
    out_d = nc.dram_tensor("out_s", [MS, NS], bf16, kind="ExternalOutput")

    with tile.TileContext(nc) as tc:
        with (
            tc.tile_pool(name="const", bufs=1) as const,
            tc.tile_pool(name="wpool", bufs=2) as wpool,
            tc.tile_pool(name="w8pool", bufs=2) as w8pool,
            tc.tile_pool(name="xp", bufs=6) as xp,
            tc.tile_pool(name="x8p", bufs=6) as x8p,
            tc.tile_pool(name="zp", bufs=2) as zp,
            tc.tile_pool(name="tp", bufs=2) as tp,
            tc.tile_pool(name="otp", bufs=2) as otp,
            tc.tile_pool(name="ps", bufs=8, space="PSUM") as psp,
        ):
            v_sb = const.tile([P, MP], f32, tag="vsb")
            u_b = const.tile([P, NS], f32, tag="ub")
            b_b = const.tile([P, NS], f32, tag="bb")
            s_c = const.tile([P, 1], f32, tag="sc")
            nc.any.memset(s_c[:], sinv)

            # iteration-invariant small inputs: load once, before the loop.
            # Reloading them per chained iteration WAR-serializes their DMA
            # queue on the previous iteration's last eviction.
            with nc.allow_non_contiguous_dma(reason="strided/broadcast consts"):
                nc.sync.dma_start(v_sb[:], v_d[:].rearrange("(pm m) -> m pm", m=P))
                nc.sync.dma_start(u_b[:], u_d[None, :].to_broadcast([P, NS]))
                nc.sync.dma_start(b_b[:], b_d[None, :].to_broadcast([P, NS]))

            q = KB // 4  # x quarter (768 cols)

            for _ in range(loops):
                wt = wpool.tile([P, KOB * NS], bf16, tag="w")
                w8t = w8pool.tile([P, 2, JD * NS], fp8, tag="w8")

                def w_kos(a, b):
                    nc.sync.dma_start(wt[:, a * NS : b * NS], wm_d[:, a * NS : b * NS])

                def xpart(xt, pm, a, b):
                    nc.sync.dma_start(xt[:, a:b], xt_d[pm * P : (pm + 1) * P, a:b])

                def x8load(xt8, pm):
                    nc.sync.dma_start(
                        xt8[:],
                        x8_d[pm * P : (pm + 1) * P, :].rearrange(
                            "p (i m) -> p i m", i=2
                        ),
                    )

                # -- head: deadline-ordered stream for the panel-0/1 pair --
                xt0 = xp.tile([P, KB], bf16, tag="xt")
                xt1 = xp.tile([P, KB], bf16, tag="xt")
                xpart(xt0, 0, 0, q)
                xpart(xt1, 1, 0, q)
                wchunks = [(0, 1), (1, 2)] + [
                    (a, min(a + 2, KOB)) for a in range(2, KOB, 2)
                ]
                nq = len(wchunks)
                for idx, (a, b) in enumerate(wchunks):
                    w_kos(a, b)
                    if idx == nq // 4:
                        xpart(xt0, 0, q, 2 * q)
                        xpart(xt1, 1, q, 2 * q)
                    elif idx == nq // 2:
                        xpart(xt0, 0, 2 * q, 3 * q)
                        xpart(xt1, 1, 2 * q, 3 * q)
                    elif idx == 3 * nq // 4:
                        xpart(xt0, 0, 3 * q, KB)
                        xpart(xt1, 1, 3 * q, KB)
                # fp8 weights + pair fp8 x ride behind the bf16 w stream
                nc.sync.dma_start(
                    w8t[:], w8_d[:].rearrange("p (i n) -> p i n", i=2)
                )
                xt8_0 = x8p.tile([P, 2, JD * P], fp8, tag="xt8")
                xt8_1 = x8p.tile([P, 2, JD * P], fp8, tag="xt8")
                x8load(xt8_0, 0)
                x8load(xt8_1, 1)
                # panel 2's data + broadcast constants ride the tail
                xt2 = xp.tile([P, KB], bf16, tag="xt")
                xt8_2 = x8p.tile([P, 2, JD * P], fp8, tag="xt8")
                xpart(xt2, 2, 0, q)
                x8load(xt8_2, 2)

                def w_slice(ko, nt):
                    base = ko * NS + nt * NFREE
                    return wt[:, base : base + NFREE]

                def w8_slice(j, nt):
                    base = j * NS + nt * NFREE
                    return w8t[:, :, base : base + NFREE]

                def dr(ps8, xt8, j, nt, start, stop):
                    nc.tensor.matmul(
                        ps8[:],
                        xt8[:, :, j * P : (j + 1) * P],
                        w8_slice(j, nt),
                        start=start, stop=stop, perf_mode=DR,
                    )

                def evict(pm, psA, psB, ps8A, ps8B):
                    # z = u*v + b on DVE (v precomputed on host)
                    z = zp.tile([P, NS], f32, tag="z")
                    nc.vector.scalar_tensor_tensor(
                        out=z[:], in0=u_b[:], scalar=v_sb[:, pm : pm + 1], in1=b_b[:],
                        op0=mult, op1=add,
                    )
                    # t = ps8 * 2^-18 + z  (can start as soon as the fp8
                    # accumulation group stopped)
                    t = tp.tile([P, NS], f32, tag="t")
                    nc.vector.scalar_tensor_tensor(
                        out=t[:, 0:NFREE], in0=ps8A[:], scalar=s_c[:, 0:1],
                        in1=z[:, 0:NFREE], op0=mult, op1=add,
                    )
                    nc.vector.scalar_tensor_tensor(
                        out=t[:, NFREE:NS], in0=ps8B[:], scalar=s_c[:, 0:1],
                        in1=z[:, NFREE:NS], op0=mult, op1=add,
                    )
                    ot = otp.tile([P, NS], bf16, tag="ot")
                    rows = slice(pm * P, (pm + 1) * P)
                    nc.vector.tensor_add(ot[:, 0:NFREE], psA[:], t[:, 0:NFREE])
                    nc.vector.tensor_add(ot[:, NFREE:NS], psB[:], t[:, NFREE:NS])
                    nc.scalar.activation(ot[:], ot[:], relu)
                    nc.sync.dma_start(out_d[rows, :], ot[:])

                # ---- panels 0-1: interleaved pair (w still streaming) ----
                ps00 = psp.tile([P, NFREE], f32, tag="ps")
                ps01 = psp.tile([P, NFREE], f32, tag="ps")
                ps10 = psp.tile([P, NFREE], f32, tag="ps")
                ps11 = psp.tile([P, NFREE], f32, tag="ps")
                for ko in range(KOB):
                    first = ko == 0
                    last = ko == KOB - 1
                    l0 = xt0[:, ko * P : (ko + 1) * P]
                    l1 = xt1[:, ko * P : (ko + 1) * P]
                    nc.tensor.matmul(ps00[:], l0, w_slice(ko, 0), start=first, stop=last)
                    nc.tensor.matmul(ps01[:], l0, w_slice(ko, 1), start=first, stop=last)
                    nc.tensor.matmul(ps10[:], l1, w_slice(ko, 0), start=first, stop=last)
                    nc.tensor.matmul(ps11[:], l1, w_slice(ko, 1), start=first, stop=last)
                # fp8 DR instrs close out the pair's accumulation (w8 arrived
                # during the bf16 stream)
                ps8_00 = psp.tile([P, NFREE], f32, tag="ps")
                ps8_01 = psp.tile([P, NFREE], f32, tag="ps")
                ps8_10 = psp.tile([P, NFREE], f32, tag="ps")
                ps8_11 = psp.tile([P, NFREE], f32, tag="ps")
                for j in range(JD):
                    first = j == 0
                    last = j == JD - 1
                    dr(ps8_00, xt8_0, j, 0, first, last)
                    dr(ps8_01, xt8_0, j, 1, first, last)
                    dr(ps8_10, xt8_1, j, 0, first, last)
                    dr(ps8_11, xt8_1, j, 1, first, last)
                # finish panel 2, stage panel 3 behind the evictions
                xpart(xt2, 2, q, KB)
                pre_x = {2: (xt2, xt8_2)}
                for pp in (3, 4, 5):
                    xtp = xp.tile([P, KB], bf16, tag="xt")
                    xt8p = x8p.tile([P, 2, JD * P], fp8, tag="xt8")
                    xpart(xtp, pp, 0, KB)
                    x8load(xt8p, pp)
                    pre_x[pp] = (xtp, xt8p)
                evict(0, ps00, ps01, ps8_00, ps8_01)
                evict(1, ps10, ps11, ps8_10, ps8_11)

                # ---- panels 2-15: solo (w resident) ----
                for pm in range(2, MP):
                    xt, xt8 = pre_x.pop(pm)
                    if pm + 4 < MP:
                        nxt = xp.tile([P, KB], bf16, tag="xt")
                        nxt8 = x8p.tile([P, 2, JD * P], fp8, tag="xt8")
                        xpart(nxt, pm + 4, 0, KB)
                        x8load(nxt8, pm + 4)
                        pre_x[pm + 4] = (nxt, nxt8)
                    psA = psp.tile([P, NFREE], f32, tag="ps")
                    psB = psp.tile([P, NFREE], f32, tag="ps")
                    ps8A = psp.tile([P, NFREE], f32, tag="ps")
                    ps8B = psp.tile([P, NFREE], f32, tag="ps")
                    # fp8 groups first: their banks retire early into t
                    for nt, ps8 in ((0, ps8A), (1, ps8B)):
                        for j in range(JD):
                            dr(ps8, xt8, j, nt, j == 0, j == JD - 1)
                    for nt, ps in ((0, psA), (1, psB)):
                        for ko in range(KOB):
                            lh = xt[:, ko * P : (ko + 1) * P]
                            nc.tensor.matmul(
                                ps[:], lh, w_slice(ko, nt),
                                start=(ko == 0), stop=(ko == KOB - 1),
                            )
                    evict(pm, psA, psB, ps8A, ps8B)

    nc.compile()
    return nc


def get_nc(variant="rank1", loops=1):
    sw = SW if variant == "rank1" else SW_GEN
    key = (loops, sw)
    if key not in _NC_CACHE:
        _NC_CACHE[key] = _build(loops, sw)
    return _NC_CACHE[key]


def pick_variant(w_sigma):
    w_sigma = np.asarray(w_sigma)
    return "rank1" if bool((w_sigma == w_sigma[0:1, :]).all()) else "general"


def _to_bf16(a):
    import ml_dtypes

    return np.ascontiguousarray(a).astype(ml_dtypes.bfloat16)


def _to_fp8(a, scale):
    import ml_dtypes

    s = np.clip(np.asarray(a, dtype=np.float32) * scale, -240.0, 240.0)
    return np.ascontiguousarray(s).astype(ml_dtypes.float8_e4m3)


def _xt_layout(xs):
    # [MS, KB] -> xt[pm*128+ki, ko*128+m] = xs[pm*128+m, ko*128+ki]
    a = xs.reshape(MP, P, KOB, P)          # [pm, m, ko, ki]
    return a.transpose(0, 3, 2, 1).reshape(MS, KB)


def _x8_layout(xs8):
    # [MS, K8] (fp8 values) -> x8[pm*128+p, i*JD*128 + j*128 + m]
    #   = xs8[pm*128+m, j*256 + i*128 + p]
    a = xs8.reshape(MP, P, JD, 2, P)       # [pm, m, j, i, p]
    return a.transpose(0, 4, 3, 2, 1).reshape(MS, K8)


def _w_layout(ws):
    # [KB, NS] -> wm[ki, ko*NS+n] = ws[ko*128+ki, n]
    return ws.reshape(KOB, P, NS).transpose(1, 0, 2).reshape(P, KOB * NS)


def _w8_layout(ws8):
    # [K8, NS] (fp8 values) -> w8[p, i*JD*NS + j*NS + n] = ws8[j*256+i*128+p, n]
    a = ws8.reshape(JD, 2, P, NS)          # [j, i, p, n]
    return a.transpose(2, 1, 0, 3).reshape(P, 2 * JD * NS)


def shard_inputs(x, w_mu, w_sigma, b_mu, b_sigma, eps_in, eps_out, variant="rank1"):
    x = np.asarray(x, dtype=np.float32)
    w_mu = np.asarray(w_mu, dtype=np.float32)
    w_sigma = np.asarray(w_sigma, dtype=np.float32)
    b_mu = np.asarray(b_mu, dtype=np.float32)
    b_sigma = np.asarray(b_sigma, dtype=np.float32)
    eps_in = np.asarray(eps_in, dtype=np.float32)
    eps_out = np.asarray(eps_out, dtype=np.float32)

    # v = x @ eps_in per batch row-group (tiny rank-1 preprocessing)
    vs = [
        np.ascontiguousarray(x[mr * MS : (mr + 1) * MS, :] @ eps_in, dtype=np.float32)
        for mr in range(MSHARDS)
    ]
    # one pre-transposed bf16 x + one fp8 x tail per batch row-group,
    # shared by 4 cores each
    xts = [
        _to_bf16(_xt_layout(x[mr * MS : (mr + 1) * MS, 0:KB]))
        for mr in range(MSHARDS)
    ]
    x8s = [
        _x8_layout(_to_fp8(x[mr * MS : (mr + 1) * MS, KB:IN_DIM], SX))
        for mr in range(MSHARDS)
    ]

    in_maps = []
    for c in range(MSHARDS * NSHARDS):
        mr, ncol = divmod(c, NSHARDS)
        nsl = slice(ncol * NS, (ncol + 1) * NS)
        if variant == "rank1":
            wshard = w_mu[:, nsl]
            u = w_sigma[0, nsl] * eps_out[nsl]
            sw = SW
        else:
            # general fallback: materialize noisy W on host, disable rank-1 term
            wshard = w_mu[:, nsl] + w_sigma[:, nsl] * (
                eps_in[:, None] * eps_out[None, nsl]
            )
            u = np.zeros(NS, dtype=np.float32)
            sw = SW_GEN
        m = {
            "xt_s": xts[mr],
            "x8_s": x8s[mr],
            "wm_s": _to_bf16(_w_layout(wshard[0:KB, :])),
            "w8_s": _w8_layout(_to_fp8(wshard[KB:IN_DIM, :], sw)),
            "u_s": np.ascontiguousarray(u, dtype=np.float32),
            "b_s": np.ascontiguousarray(
                b_mu[nsl] + b_sigma[nsl] * eps_out[nsl], dtype=np.float32
            ),
            "v_s": vs[mr],
        }
        in_maps.append(m)
    return in_maps


def unshard_output(results):
    out = np.empty((BATCH, UNITS), dtype=np.float32)
    for c, rmap in enumerate(results):
        mr, ncol = divmod(c, NSHARDS)
        out[mr * MS : (mr + 1) * MS, ncol * NS : (ncol + 1) * NS] = np.asarray(
            rmap["out_s"]
        ).astype(np.float32)
    return out


def kernel(x, w_mu, w_sigma, b_mu, b_sigma, eps_in, eps_out):
    from concourse.bass_utils import run_bass_kernel_spmd

    variant = pick_variant(w_sigma)
    nc = get_nc(variant)
    in_maps = shard_inputs(
        x, w_mu, w_sigma, b_mu, b_sigma, eps_in, eps_out, variant=variant
    )
    res = run_bass_kernel_spmd(nc, in_maps, core_ids=list(range(8)))
    return unshard_output(res.results)
